# revision 30
# baseline (speedup 1.0000x reference)
"""Multi-head attention (B=2, S=2048, D=1024, H=16) on 8 TRN2 NeuronCores.

Sharding: tensor-parallel over heads (TP=4, 4 heads / 256 dims per core)
x data-parallel over batch (DP=2). Core c = 4*b + t handles batch b,
head group t. Each core computes Q/K/V projections for its heads,
attention in a transposed-scores layout (scores^T = [s_k, s_q], softmax
across partitions via a ones-column appended to V and a K=1 outer-product
broadcast of the reciprocal), then its partial output projection.
Partials are ReduceScattered over each batch's 4-core TP group; the host
reassembles the full [B, S, D] output.

All matmul operands are bf16 (fp32 PSUM accumulation); softmax
denominators/reciprocals and the output path are fp32. The key mask is
folded into the exp as a per-partition bias (0 or -60).
"""

import contextlib
import numpy as np
import ml_dtypes

import concourse.bass as bass
import concourse.tile as tile
from concourse import bacc, mybir
from concourse.bass_utils import run_bass_kernel_spmd

F32 = mybir.dt.float32
BF16 = mybir.dt.bfloat16
Exp = mybir.ActivationFunctionType.Exp

B, S, D, H = 2, 2048, 1024, 16
DK = D // H                      # 64
TP, DP = 4, 2
HPC = H // TP                    # heads per core = 4
DSH = D // TP                    # shard dims per core = 256
NPAIR = HPC // 2                 # head pairs per core = 2
QB = 512                         # query block
NQB = S // QB                    # 4
KT = 128                         # key tile
NKT = S // KT                    # 16
NKB = D // 128                   # 8 contraction tiles for projections
MASK_NEG = -60.0

REPLICA_GROUPS = [[0, 1, 2, 3], [4, 5, 6, 7]]


def build_nc(with_collective=True):
    nc = bacc.Bacc("TRN2", target_bir_lowering=False, debug=False, num_devices=DP * TP)

    # ---- parameters (per-core shards, host-prepped layouts)
    xq = nc.declare_dram_parameter("xq", [NKB, 128, S], BF16, isOutput=False)   # q_in[b].T
    xk = nc.declare_dram_parameter("xk", [NKB, 128, S], BF16, isOutput=False)
    xv = nc.declare_dram_parameter("xv", [NKB, 128, S], BF16, isOutput=False)
    # weights pre-packed on host into the exact SBUF layout -> 1 DMA each
    wq = nc.declare_dram_parameter("wq", [128, NKB * DSH], BF16, isOutput=False)
    wk = nc.declare_dram_parameter("wk", [128, NKB * DSH], BF16, isOutput=False)
    wv = nc.declare_dram_parameter("wv", [128, NKB * DSH], BF16, isOutput=False)
    wo = nc.declare_dram_parameter("wo", [128, 2 * D], BF16, isOutput=False)
    bq = nc.declare_dram_parameter("bq", [128, 2], F32, isOutput=False)
    bk = nc.declare_dram_parameter("bk", [128, 2], F32, isOutput=False)
    bvb = nc.declare_dram_parameter("bvb", [128, DSH], F32, isOutput=False)      # b_v shard bcast
    bob = nc.declare_dram_parameter("bob", [128, D], F32, isOutput=False)        # b_o bcast
    mb = nc.declare_dram_parameter("mb", [128, NKT], F32, isOutput=False)        # mask bias
    out = nc.declare_dram_parameter("out", [NQB, 128, D], F32, isOutput=True)

    with tile.TileContext(nc) as tc, contextlib.ExitStack() as ctx:
        const = ctx.enter_context(tc.tile_pool(name="const", bufs=1))
        xp = ctx.enter_context(tc.tile_pool(name="xp", bufs=3 * NKB))
        qt_p = ctx.enter_context(tc.tile_pool(name="qt", bufs=2 * NQB))
        kt_p = ctx.enter_context(tc.tile_pool(name="ktp", bufs=2 * NQB))
        vp_p = ctx.enter_context(tc.tile_pool(name="vp", bufs=NKT))
        exp_p = ctx.enter_context(tc.tile_pool(name="expp", bufs=4))
        ctx_p = ctx.enter_context(tc.tile_pool(name="ctxp", bufs=4))
        rec_p = ctx.enter_context(tc.tile_pool(name="recp", bufs=2))
        rb_p = ctx.enter_context(tc.tile_pool(name="rbp", bufs=2))
        po_p = ctx.enter_context(tc.tile_pool(name="pop", bufs=2))
        os_p = ctx.enter_context(tc.tile_pool(name="osp", bufs=2))
        ps_s = ctx.enter_context(tc.tile_pool(name="pss", bufs=2, space="PSUM"))
        ps_av = ctx.enter_context(tc.tile_pool(name="psav", bufs=2, space="PSUM"))
        ps_sm = ctx.enter_context(tc.tile_pool(name="pssm", bufs=2, space="PSUM"))
        dram = ctx.enter_context(tc.tile_pool(name="dram", bufs=2, space="DRAM"))

        # ---- constants (each one contiguous DMA; ordered by first use)
        w_sb = {name: const.tile([128, NKB * DSH], BF16, name=f"{name}_sb")
                for name in ("wk", "wv", "wq")}
        wo_sb = const.tile([128, 2 * D], BF16)
        bq_sb = const.tile([128, 2], F32)
        bk_sb = const.tile([128, 2], F32)
        bvb_sb = const.tile([128, DSH], F32)
        bob_sb = const.tile([128, D], F32)
        mb_sb = const.tile([128, NKT], F32)
        ones_sb = const.tile([128, DK], F32)
        nc.scalar.dma_start(out=w_sb["wk"][:], in_=wk[:])
        nc.scalar.dma_start(out=w_sb["wv"][:], in_=wv[:])
        nc.scalar.dma_start(out=mb_sb[:], in_=mb[:])
        nc.scalar.dma_start(out=bk_sb[:], in_=bk[:])
        nc.scalar.dma_start(out=bvb_sb[:], in_=bvb[:])
        nc.any.memset(ones_sb[:], 1.0)

        # ---- phase A: projections
        # K^T and Q^T per (pair m, s-block nb): tiles [128, 512]
        #   partitions 0:64 = head 2m dims, 64:128 = head 2m+1 dims
        # V' per s-tile st: [128, HPC*65] with ones col at 64 of each 65
        KT_t = {}
        QT_t = {}
        VP_t = {}

        _xt_cache = {}

        def load_x(xprm, wname, eng, chunked=False):
            xt = [xp.tile([128, S], BF16, name=f"x_{wname}_{kb}", tag="xtile")
                  for kb in range(NKB)]
            if chunked:
                # nb-major: the first projection chain only needs every
                # tile's first chunk, so it can start ~3x earlier
                for nb in range(NQB):
                    for kb in range(NKB):
                        eng.dma_start(out=xt[kb][:, nb * QB:(nb + 1) * QB],
                                      in_=xprm[kb, :, nb * QB:(nb + 1) * QB])
            else:
                for kb in range(NKB):
                    eng.dma_start(out=xt[kb][:], in_=xprm[kb])
            _xt_cache[wname] = xt

        def proj_qk_chain(wname, bias_sb, store, nb, m):
            xt = _xt_cache[wname]
            ps = ps_sm.tile([128, QB], F32, name=f"ps_{wname}_{m}_{nb}", tag="smps")
            for kb in range(NKB):
                nc.tensor.matmul(
                    ps[:],
                    w_sb[wname][:, kb * DSH + m * 128: kb * DSH + (m + 1) * 128],
                    xt[kb][:, nb * QB:(nb + 1) * QB],
                    start=(kb == 0), stop=(kb == NKB - 1),
                )
            dst = (qt_p if store is QT_t else kt_p).tile(
                [128, QB], BF16, name=f"{wname}t_{m}_{nb}", tag="proj")
            nc.vector.tensor_scalar_add(dst[:], ps[:], bias_sb[:, m:m + 1])
            store[(m, nb)] = dst

        def proj_v_chain(st):
            xt = _xt_cache["wv"]
            ps = ps_sm.tile([128, QB], F32, name=f"ps_v_{st}", tag="smps")[:, 0:DSH]
            for kb in range(NKB):
                nc.tensor.matmul(
                    ps[:],
                    xt[kb][:, st * 128:(st + 1) * 128],
                    w_sb["wv"][:, kb * DSH:(kb + 1) * DSH],
                    start=(kb == 0), stop=(kb == NKB - 1),
                )
            vp = vp_p.tile([128, HPC * (DK + 1)], BF16, name=f"vp_{st}", tag="vp")
            for h in range(HPC):
                col = h * (DK + 1) + DK
                nc.any.memset(vp[:, col:col + 1], 1.0)
            ps3 = ps.rearrange("p (h d) -> p h d", h=HPC)
            bv3 = bvb_sb.rearrange("p (h d) -> p h d", h=HPC)
            vp3 = vp.rearrange("p (h d) -> p h d", h=HPC)[:, :, 0:DK]
            nc.vector.tensor_add(vp3, ps3, bv3)
            VP_t[st] = vp

        # x loads: xk full tiles + xq split (nb0 chunk early, rest later)
        # on the sync ring; xv full tiles behind the K/V weights on the
        # scalar ring. Late consts trail on the scalar ring.
        xt_k = [xp.tile([128, S], BF16, name=f"x_wk_{kb}", tag="xtile")
                for kb in range(NKB)]
        xt_v = [xp.tile([128, S], BF16, name=f"x_wv_{kb}", tag="xtile")
                for kb in range(NKB)]
        xt_q = [xp.tile([128, S], BF16, name=f"x_wq_{kb}", tag="xtile")
                for kb in range(NKB)]
        _xt_cache.update(wk=xt_k, wv=xt_v, wq=xt_q)
        for kb in range(NKB):
            nc.sync.dma_start(out=xt_k[kb][:], in_=xk[kb])
        for kb in range(NKB):
            nc.scalar.dma_start(out=xt_v[kb][:], in_=xv[kb])
        for kb in range(NKB):
            nc.sync.dma_start(out=xt_q[kb][:, 0:QB], in_=xq[kb, :, 0:QB])
        nc.scalar.dma_start(out=w_sb["wq"][:], in_=wq[:])
        nc.scalar.dma_start(out=bq_sb[:], in_=bq[:])
        for kb in range(NKB):
            nc.sync.dma_start(out=xt_q[kb][:, QB:S], in_=xq[kb, :, QB:S])
        nc.scalar.dma_start(out=wo_sb[:], in_=wo[:])
        nc.scalar.dma_start(out=bob_sb[:], in_=bob[:])

        for nb in range(NQB):
            for m in range(2):
                proj_qk_chain("wk", bk_sb, KT_t, nb, m)
        for m in range(2):
            proj_qk_chain("wq", bq_sb, QT_t, 0, m)
        for st in range(2):
            proj_v_chain(st)

        # ---- phase B: attention + output projection + reduce-scatter
        for qb in range(NQB):
            ctx_pair = []
            for m in range(NPAIR):
                av = [ps_av.tile([128, QB], F32, name=f"av_{qb}_{m}_{p}", tag="av")
                      for p in range(2)]

                def emit_av(kt, ets):
                    for p in range(2):
                        h = 2 * m + p
                        nc.tensor.matmul(
                            av[p][0:DK + 1, :],
                            VP_t[kt][:, h * (DK + 1):(h + 1) * (DK + 1)],
                            ets[:, p * QB:(p + 1) * QB],
                            start=(kt == 0), stop=(kt == NKT - 1),
                        )

                # software pipeline: AV(kt-1) is emitted after scores(kt) so
                # the PE never head-of-line blocks on exp(kt)
                prev_et = None
                for kt in range(NKT):
                    nb, co = kt // 4, (kt % 4) * 128
                    pss = ps_s.tile([128, 2 * QB], F32, name=f"pss_{qb}_{m}_{kt}", tag="pss")
                    # head 2m on partitions 0:64, head 2m+1 on 64:128;
                    # different PSUM banks for the two row groups (HW req.)
                    nc.tensor.matmul(
                        pss[:, 0:QB],
                        KT_t[(m, nb)][0:64, co:co + 128],
                        QT_t[(m, qb)][0:64, :],
                        start=True, stop=True)
                    nc.tensor.matmul(
                        pss[:, QB:2 * QB],
                        KT_t[(m, nb)][64:128, co:co + 128],
                        QT_t[(m, qb)][64:128, :],
                        start=True, stop=True)
                    et = exp_p.tile([128, 2 * QB], BF16, name=f"exp_{qb}_{m}_{kt}", tag="exp")
                    nc.scalar.activation(et[:], pss[:], Exp,
                                         bias=mb_sb[:, kt:kt + 1], scale=1.0 / np.sqrt(DK))
                    if prev_et is not None:
                        emit_av(kt - 1, prev_et)
                    prev_et = et
                    # PE fillers inside the ACT-bound loop: remaining V' and
                    # K chains during (qb0, m0); next Q block during each m1.
                    # K(nb) is consumed from iteration 4*nb on; V'(st) from
                    # iteration st on.
                    if qb == 0 and m == 0:
                        if kt + 2 < NKT:
                            proj_v_chain(kt + 2)
                    if m == 1 and qb + 1 < NQB and kt in (4, 12):
                        proj_qk_chain("wq", bq_sb, QT_t, qb + 1, 0 if kt == 4 else 1)
                emit_av(NKT - 1, prev_et)
                cpt = ctx_p.tile([128, QB], BF16, name=f"ctx_{qb}_{m}", tag="ctx")
                for p in range(2):
                    rec = rec_p.tile([128, QB], F32, name=f"rec_{qb}_{m}_{p}", tag="rec")
                    nc.vector.reciprocal(rec[64:65, :], av[p][DK:DK + 1, :])
                    rbp = ps_sm.tile([128, QB], F32, name=f"rbp_{qb}_{m}_{p}", tag="smps")
                    nc.tensor.matmul(rbp[0:DK, :], ones_sb[64:65, :],
                                     rec[64:65, :], start=True, stop=True)
                    rbs = rb_p.tile([DK, QB], F32, name=f"rbs_{qb}_{m}_{p}", tag="rbs")
                    nc.vector.tensor_copy(rbs[:], rbp[0:DK, :])
                    nc.vector.tensor_mul(cpt[p * DK:(p + 1) * DK, :], av[p][0:DK, :], rbs[:])
                ctx_pair.append(cpt)

            partial = dram.tile([QB, D], F32, name=f"partial_{qb}", tag="partial")
            for st in range(NQB):
                for dh in range(2):
                    pso = ps_sm.tile([128, 512], F32, name=f"pso_{qb}_{st}_{dh}", tag="smps")
                    for m in range(NPAIR):
                        nc.tensor.matmul(
                            pso[:],
                            ctx_pair[m][:, st * 128:(st + 1) * 128],
                            wo_sb[:, m * D + dh * 512: m * D + (dh + 1) * 512],
                            start=(m == 0), stop=(m == NPAIR - 1),
                        )
                    pos = po_p.tile([128, 512], F32, name=f"pos_{qb}_{st}_{dh}", tag="pos")
                    nc.vector.tensor_copy(pos[:], pso[:])
                    nc.sync.dma_start(
                        out=partial[st * 128:(st + 1) * 128, dh * 512:(dh + 1) * 512],
                        in_=pos[:])

            rs_out = dram.tile([128, D], F32, name=f"rs_{qb}", tag="rs")
            if with_collective:
                nc.gpsimd.collective_compute(
                    "ReduceScatter", mybir.AluOpType.add,
                    replica_groups=REPLICA_GROUPS,
                    ins=[partial[:].opt()], outs=[rs_out[:].opt()])
            else:
                nc.sync.dma_start(out=rs_out[:], in_=partial[0:128, :])
            osb = os_p.tile([128, D], F32, name=f"os_{qb}", tag="os")
            nc.sync.dma_start(out=osb[:], in_=rs_out[:])
            fin = os_p.tile([128, D], F32, name=f"fin_{qb}", tag="fin")
            nc.vector.tensor_add(fin[:], osb[:], bob_sb[:])
            nc.sync.dma_start(out=out[qb], in_=fin[:])

    nc.compile()
    return nc


def _prep_inputs(q_in, k_in, v_in, mask, w_q, b_q, w_k, b_k, w_v, b_v, w_o, b_o):
    BF = ml_dtypes.bfloat16
    xq_b, xk_b, xv_b, mb_b = [], [], [], []
    for b in range(B):
        xq_b.append(np.ascontiguousarray(q_in[b].T).astype(BF).reshape(NKB, 128, S))
        xk_b.append(np.ascontiguousarray(k_in[b].T).astype(BF).reshape(NKB, 128, S))
        xv_b.append(np.ascontiguousarray(v_in[b].T).astype(BF).reshape(NKB, 128, S))
        mbias = ((mask[b, 0, 0, :] == 0) * np.float32(MASK_NEG)).astype(np.float32)
        mb_b.append(np.ascontiguousarray(mbias.reshape(NKT, 128).T))
    bob = np.ascontiguousarray(np.broadcast_to(b_o.astype(np.float32), (128, D)))
    in_maps = []
    for c in range(DP * TP):
        b, t = c // TP, c % TP
        sl = slice(DSH * t, DSH * (t + 1))
        def pack_w(w_t, nblk):
            # [d_in, cols] -> SBUF layout [128, nblk*cols]: block kb at
            # columns [kb*cols:(kb+1)*cols] holds d_in rows kb*128..+128
            cols = w_t.shape[1]
            return np.ascontiguousarray(
                w_t.reshape(nblk, 128, cols).transpose(1, 0, 2).reshape(128, nblk * cols)
            ).astype(BF)

        in_maps.append({
            "xq": xq_b[b], "xk": xk_b[b], "xv": xv_b[b],
            "wq": pack_w(np.ascontiguousarray(w_q[sl, :].T), NKB),
            "wk": pack_w(np.ascontiguousarray(w_k[sl, :].T), NKB),
            "wv": pack_w(np.ascontiguousarray(w_v[sl, :].T), NKB),
            "wo": pack_w(np.ascontiguousarray(w_o[:, sl].T), 2),
            "bq": np.ascontiguousarray(b_q[sl].astype(np.float32).reshape(2, 128).T),
            "bk": np.ascontiguousarray(b_k[sl].astype(np.float32).reshape(2, 128).T),
            "bvb": np.ascontiguousarray(
                np.broadcast_to(b_v[sl].astype(np.float32), (128, DSH))),
            "bob": bob,
            "mb": mb_b[b],
        })
    return in_maps


_NC_CACHE = {}


def kernel(q_in, k_in, v_in, mask, w_q, b_q, w_k, b_k, w_v, b_v, w_o, b_o):
    if "nc" not in _NC_CACHE:
        _NC_CACHE["nc"] = build_nc()
    nc = _NC_CACHE["nc"]
    in_maps = _prep_inputs(q_in, k_in, v_in, mask,
                           w_q, b_q, w_k, b_k, w_v, b_v, w_o, b_o)
    res = run_bass_kernel_spmd(nc, in_maps, list(range(DP * TP))).results
    full = np.empty((B, S, D), np.float32)
    for b in range(B):
        for r in range(TP):
            o = res[TP * b + r]["out"]          # [NQB, 128, D]
            for qb in range(NQB):
                row = qb * QB + r * 128
                full[b, row:row + 128] = o[qb]
    return full


# revision 32
# speedup vs baseline: 400.6050x; 400.6050x over previous
"""Multi-head attention (B=2, S=2048, D=1024, H=16) on 8 TRN2 NeuronCores.

Sharding: tensor-parallel over heads (TP=4, 4 heads / 256 dims per core)
x data-parallel over batch (DP=2). Core c = 4*b + t handles batch b,
head group t. Each core computes Q/K/V projections for its heads,
attention in a transposed-scores layout (scores^T = [s_k, s_q], softmax
across partitions via a ones-column appended to V and a K=1 outer-product
broadcast of the reciprocal), then its partial output projection.
Partials are ReduceScattered over each batch's 4-core TP group; the host
reassembles the full [B, S, D] output.

All matmul operands are bf16 (fp32 PSUM accumulation); softmax
denominators/reciprocals and the output path are fp32. The key mask is
folded into the exp as a per-partition bias (0 or -60).
"""

import contextlib
import numpy as np
import ml_dtypes

import concourse.bass as bass
import concourse.tile as tile
from concourse import bacc, mybir
from concourse.bass_utils import run_bass_kernel_spmd

F32 = mybir.dt.float32
BF16 = mybir.dt.bfloat16
Exp = mybir.ActivationFunctionType.Exp

B, S, D, H = 2, 2048, 1024, 16
DK = D // H                      # 64
TP, DP = 4, 2
HPC = H // TP                    # heads per core = 4
DSH = D // TP                    # shard dims per core = 256
NPAIR = HPC // 2                 # head pairs per core = 2
QB = 512                         # query block
NQB = S // QB                    # 4
KT = 128                         # key tile
NKT = S // KT                    # 16
NKB = D // 128                   # 8 contraction tiles for projections
MASK_NEG = -60.0

REPLICA_GROUPS = [[0, 1, 2, 3], [4, 5, 6, 7]]


def build_nc(with_collective=True):
    nc = bacc.Bacc("TRN2", target_bir_lowering=False, debug=False, num_devices=DP * TP)

    # ---- parameters (per-core shards, host-prepped layouts)
    xq = nc.declare_dram_parameter("xq", [NKB, 128, S], BF16, isOutput=False)   # q_in[b].T
    xk = nc.declare_dram_parameter("xk", [NKB, 128, S], BF16, isOutput=False)
    xv = nc.declare_dram_parameter("xv", [NKB, 128, S], BF16, isOutput=False)
    # weights pre-packed on host into the exact SBUF layout -> 1 DMA each
    wq = nc.declare_dram_parameter("wq", [128, NKB * DSH], BF16, isOutput=False)
    wk = nc.declare_dram_parameter("wk", [128, NKB * DSH], BF16, isOutput=False)
    wv = nc.declare_dram_parameter("wv", [128, NKB * DSH], BF16, isOutput=False)
    wo = nc.declare_dram_parameter("wo", [128, 2 * D], BF16, isOutput=False)
    bq = nc.declare_dram_parameter("bq", [128, 2], F32, isOutput=False)
    bk = nc.declare_dram_parameter("bk", [128, 2], F32, isOutput=False)
    bvb = nc.declare_dram_parameter("bvb", [128, DSH], F32, isOutput=False)      # b_v shard bcast
    bob = nc.declare_dram_parameter("bob", [128, D], F32, isOutput=False)        # b_o bcast
    mb = nc.declare_dram_parameter("mb", [128, NKT], F32, isOutput=False)        # mask bias
    out = nc.declare_dram_parameter("out", [NQB, 128, D], F32, isOutput=True)

    with tile.TileContext(nc) as tc, contextlib.ExitStack() as ctx:
        const = ctx.enter_context(tc.tile_pool(name="const", bufs=1))
        xp = ctx.enter_context(tc.tile_pool(name="xp", bufs=3 * NKB))
        qt_p = ctx.enter_context(tc.tile_pool(name="qt", bufs=2 * NQB))
        kt_p = ctx.enter_context(tc.tile_pool(name="ktp", bufs=2 * NQB))
        vp_p = ctx.enter_context(tc.tile_pool(name="vp", bufs=NKT))
        exp_p = ctx.enter_context(tc.tile_pool(name="expp", bufs=4))
        ctx_p = ctx.enter_context(tc.tile_pool(name="ctxp", bufs=4))
        rec_p = ctx.enter_context(tc.tile_pool(name="recp", bufs=2))
        rb_p = ctx.enter_context(tc.tile_pool(name="rbp", bufs=2))
        po_p = ctx.enter_context(tc.tile_pool(name="pop", bufs=2))
        os_p = ctx.enter_context(tc.tile_pool(name="osp", bufs=2))
        ps_s = ctx.enter_context(tc.tile_pool(name="pss", bufs=2, space="PSUM"))
        ps_av = ctx.enter_context(tc.tile_pool(name="psav", bufs=2, space="PSUM"))
        ps_sm = ctx.enter_context(tc.tile_pool(name="pssm", bufs=2, space="PSUM"))
        dram = ctx.enter_context(tc.tile_pool(name="dram", bufs=2, space="DRAM"))

        # ---- constants (each one contiguous DMA; ordered by first use)
        w_sb = {name: const.tile([128, NKB * DSH], BF16, name=f"{name}_sb")
                for name in ("wk", "wv", "wq")}
        wo_sb = const.tile([128, 2 * D], BF16)
        bq_sb = const.tile([128, 2], F32)
        bk_sb = const.tile([128, 2], F32)
        bvb_sb = const.tile([128, DSH], F32)
        bob_sb = const.tile([128, D], F32)
        mb_sb = const.tile([128, NKT], F32)
        ones_sb = const.tile([128, DK], F32)
        nc.scalar.dma_start(out=w_sb["wk"][:], in_=wk[:])
        nc.scalar.dma_start(out=w_sb["wv"][:], in_=wv[:])
        nc.scalar.dma_start(out=mb_sb[:], in_=mb[:])
        nc.any.memset(ones_sb[:], 1.0)

        # ---- phase A: projections
        # K^T and Q^T per (pair m, s-block nb): tiles [128, 512]
        #   partitions 0:64 = head 2m dims, 64:128 = head 2m+1 dims
        # V' per s-tile st: [128, HPC*65] with ones col at 64 of each 65
        KT_t = {}
        QT_t = {}
        VP_t = {}

        _xt_cache = {}

        def load_x(xprm, wname, eng, chunked=False):
            xt = [xp.tile([128, S], BF16, name=f"x_{wname}_{kb}", tag="xtile")
                  for kb in range(NKB)]
            if chunked:
                # nb-major: the first projection chain only needs every
                # tile's first chunk, so it can start ~3x earlier
                for nb in range(NQB):
                    for kb in range(NKB):
                        eng.dma_start(out=xt[kb][:, nb * QB:(nb + 1) * QB],
                                      in_=xprm[kb, :, nb * QB:(nb + 1) * QB])
            else:
                for kb in range(NKB):
                    eng.dma_start(out=xt[kb][:], in_=xprm[kb])
            _xt_cache[wname] = xt

        def proj_qk_chain(wname, bias_sb, store, nb, m):
            xt = _xt_cache[wname]
            ps = ps_sm.tile([128, QB], F32, name=f"ps_{wname}_{m}_{nb}", tag="smps")
            for kb in range(NKB):
                nc.tensor.matmul(
                    ps[:],
                    w_sb[wname][:, kb * DSH + m * 128: kb * DSH + (m + 1) * 128],
                    xt[kb][:, nb * QB:(nb + 1) * QB],
                    start=(kb == 0), stop=(kb == NKB - 1),
                )
            dst = (qt_p if store is QT_t else kt_p).tile(
                [128, QB], BF16, name=f"{wname}t_{m}_{nb}", tag="proj")
            nc.vector.tensor_scalar_add(dst[:], ps[:], bias_sb[:, m:m + 1])
            store[(m, nb)] = dst

        def proj_v_chain(st):
            xt = _xt_cache["wv"]
            ps = ps_sm.tile([128, QB], F32, name=f"ps_v_{st}", tag="smps")[:, 0:DSH]
            for kb in range(NKB):
                nc.tensor.matmul(
                    ps[:],
                    xt[kb][:, st * 128:(st + 1) * 128],
                    w_sb["wv"][:, kb * DSH:(kb + 1) * DSH],
                    start=(kb == 0), stop=(kb == NKB - 1),
                )
            vp = vp_p.tile([128, HPC * (DK + 1)], BF16, name=f"vp_{st}", tag="vp")
            for h in range(HPC):
                col = h * (DK + 1) + DK
                nc.any.memset(vp[:, col:col + 1], 1.0)
            ps3 = ps.rearrange("p (h d) -> p h d", h=HPC)
            bv3 = bvb_sb.rearrange("p (h d) -> p h d", h=HPC)
            vp3 = vp.rearrange("p (h d) -> p h d", h=HPC)[:, :, 0:DK]
            nc.vector.tensor_add(vp3, ps3, bv3)
            VP_t[st] = vp

        # x loads: xk full tiles + xq split (nb0 chunk early, rest later)
        # on the sync ring; xv full tiles behind the K/V weights on the
        # scalar ring. Late consts trail on the scalar ring.
        xt_k = [xp.tile([128, S], BF16, name=f"x_wk_{kb}", tag="xtile")
                for kb in range(NKB)]
        xt_v = [xp.tile([128, S], BF16, name=f"x_wv_{kb}", tag="xtile")
                for kb in range(NKB)]
        xt_q = [xp.tile([128, S], BF16, name=f"x_wq_{kb}", tag="xtile")
                for kb in range(NKB)]
        _xt_cache.update(wk=xt_k, wv=xt_v, wq=xt_q)
        for kb in range(NKB):
            nc.sync.dma_start(out=xt_k[kb][:], in_=xk[kb])
        for kb in range(NKB):
            nc.scalar.dma_start(out=xt_v[kb][:], in_=xv[kb])
        nc.scalar.dma_start(out=bk_sb[:], in_=bk[:])
        nc.scalar.dma_start(out=bvb_sb[:], in_=bvb[:])
        for kb in range(NKB):
            nc.sync.dma_start(out=xt_q[kb][:, 0:QB], in_=xq[kb, :, 0:QB])
        nc.scalar.dma_start(out=w_sb["wq"][:], in_=wq[:])
        nc.scalar.dma_start(out=bq_sb[:], in_=bq[:])
        for kb in range(NKB):
            nc.sync.dma_start(out=xt_q[kb][:, QB:S], in_=xq[kb, :, QB:S])
        nc.scalar.dma_start(out=wo_sb[:], in_=wo[:])
        nc.scalar.dma_start(out=bob_sb[:], in_=bob[:])

        for nb in range(NQB):
            for m in range(2):
                proj_qk_chain("wk", bk_sb, KT_t, nb, m)
        for m in range(2):
            proj_qk_chain("wq", bq_sb, QT_t, 0, m)
        for st in range(2):
            proj_v_chain(st)

        # ---- phase B: attention + output projection + reduce-scatter
        for qb in range(NQB):
            ctx_pair = []
            for m in range(NPAIR):
                av = [ps_av.tile([128, QB], F32, name=f"av_{qb}_{m}_{p}", tag="av")
                      for p in range(2)]

                def emit_av(kt, ets):
                    for p in range(2):
                        h = 2 * m + p
                        nc.tensor.matmul(
                            av[p][0:DK + 1, :],
                            VP_t[kt][:, h * (DK + 1):(h + 1) * (DK + 1)],
                            ets[:, p * QB:(p + 1) * QB],
                            start=(kt == 0), stop=(kt == NKT - 1),
                        )

                # software pipeline: AV(kt-1) is emitted after scores(kt) so
                # the PE never head-of-line blocks on exp(kt)
                prev_et = None
                for kt in range(NKT):
                    nb, co = kt // 4, (kt % 4) * 128
                    pss = ps_s.tile([128, 2 * QB], F32, name=f"pss_{qb}_{m}_{kt}", tag="pss")
                    # head 2m on partitions 0:64, head 2m+1 on 64:128;
                    # different PSUM banks for the two row groups (HW req.)
                    nc.tensor.matmul(
                        pss[:, 0:QB],
                        KT_t[(m, nb)][0:64, co:co + 128],
                        QT_t[(m, qb)][0:64, :],
                        start=True, stop=True)
                    nc.tensor.matmul(
                        pss[:, QB:2 * QB],
                        KT_t[(m, nb)][64:128, co:co + 128],
                        QT_t[(m, qb)][64:128, :],
                        start=True, stop=True)
                    et = exp_p.tile([128, 2 * QB], BF16, name=f"exp_{qb}_{m}_{kt}", tag="exp")
                    nc.scalar.activation(et[:], pss[:], Exp,
                                         bias=mb_sb[:, kt:kt + 1], scale=1.0 / np.sqrt(DK))
                    if prev_et is not None:
                        emit_av(kt - 1, prev_et)
                    prev_et = et
                    # PE fillers inside the ACT-bound loop: remaining V' and
                    # K chains during (qb0, m0); next Q block during each m1.
                    # K(nb) is consumed from iteration 4*nb on; V'(st) from
                    # iteration st on.
                    if qb == 0 and m == 0:
                        if kt + 2 < NKT:
                            proj_v_chain(kt + 2)
                    if m == 1 and qb + 1 < NQB and kt in (4, 12):
                        proj_qk_chain("wq", bq_sb, QT_t, qb + 1, 0 if kt == 4 else 1)
                emit_av(NKT - 1, prev_et)
                cpt = ctx_p.tile([128, QB], BF16, name=f"ctx_{qb}_{m}", tag="ctx")
                for p in range(2):
                    rec = rec_p.tile([128, QB], F32, name=f"rec_{qb}_{m}_{p}", tag="rec")
                    nc.vector.reciprocal(rec[64:65, :], av[p][DK:DK + 1, :])
                    rbp = ps_sm.tile([128, QB], F32, name=f"rbp_{qb}_{m}_{p}", tag="smps")
                    nc.tensor.matmul(rbp[0:DK, :], ones_sb[64:65, :],
                                     rec[64:65, :], start=True, stop=True)
                    rbs = rb_p.tile([DK, QB], F32, name=f"rbs_{qb}_{m}_{p}", tag="rbs")
                    nc.vector.tensor_copy(rbs[:], rbp[0:DK, :])
                    nc.vector.tensor_mul(cpt[p * DK:(p + 1) * DK, :], av[p][0:DK, :], rbs[:])
                ctx_pair.append(cpt)

            partial = dram.tile([QB, D], F32, name=f"partial_{qb}", tag="partial")
            for st in range(NQB):
                for dh in range(2):
                    pso = ps_sm.tile([128, 512], F32, name=f"pso_{qb}_{st}_{dh}", tag="smps")
                    for m in range(NPAIR):
                        nc.tensor.matmul(
                            pso[:],
                            ctx_pair[m][:, st * 128:(st + 1) * 128],
                            wo_sb[:, m * D + dh * 512: m * D + (dh + 1) * 512],
                            start=(m == 0), stop=(m == NPAIR - 1),
                        )
                    pos = po_p.tile([128, 512], F32, name=f"pos_{qb}_{st}_{dh}", tag="pos")
                    nc.vector.tensor_copy(pos[:], pso[:])
                    nc.sync.dma_start(
                        out=partial[st * 128:(st + 1) * 128, dh * 512:(dh + 1) * 512],
                        in_=pos[:])

            rs_out = dram.tile([128, D], F32, name=f"rs_{qb}", tag="rs")
            if with_collective:
                nc.gpsimd.collective_compute(
                    "ReduceScatter", mybir.AluOpType.add,
                    replica_groups=REPLICA_GROUPS,
                    ins=[partial[:].opt()], outs=[rs_out[:].opt()])
            else:
                nc.sync.dma_start(out=rs_out[:], in_=partial[0:128, :])
            osb = os_p.tile([128, D], F32, name=f"os_{qb}", tag="os")
            nc.sync.dma_start(out=osb[:], in_=rs_out[:])
            fin = os_p.tile([128, D], F32, name=f"fin_{qb}", tag="fin")
            nc.vector.tensor_add(fin[:], osb[:], bob_sb[:])
            nc.sync.dma_start(out=out[qb], in_=fin[:])

    nc.compile()
    return nc


def _prep_inputs(q_in, k_in, v_in, mask, w_q, b_q, w_k, b_k, w_v, b_v, w_o, b_o):
    BF = ml_dtypes.bfloat16
    xq_b, xk_b, xv_b, mb_b = [], [], [], []
    for b in range(B):
        xq_b.append(np.ascontiguousarray(q_in[b].T).astype(BF).reshape(NKB, 128, S))
        xk_b.append(np.ascontiguousarray(k_in[b].T).astype(BF).reshape(NKB, 128, S))
        xv_b.append(np.ascontiguousarray(v_in[b].T).astype(BF).reshape(NKB, 128, S))
        mbias = ((mask[b, 0, 0, :] == 0) * np.float32(MASK_NEG)).astype(np.float32)
        mb_b.append(np.ascontiguousarray(mbias.reshape(NKT, 128).T))
    bob = np.ascontiguousarray(np.broadcast_to(b_o.astype(np.float32), (128, D)))
    in_maps = []
    for c in range(DP * TP):
        b, t = c // TP, c % TP
        sl = slice(DSH * t, DSH * (t + 1))
        def pack_w(w_t, nblk):
            # [d_in, cols] -> SBUF layout [128, nblk*cols]: block kb at
            # columns [kb*cols:(kb+1)*cols] holds d_in rows kb*128..+128
            cols = w_t.shape[1]
            return np.ascontiguousarray(
                w_t.reshape(nblk, 128, cols).transpose(1, 0, 2).reshape(128, nblk * cols)
            ).astype(BF)

        in_maps.append({
            "xq": xq_b[b], "xk": xk_b[b], "xv": xv_b[b],
            "wq": pack_w(np.ascontiguousarray(w_q[sl, :].T), NKB),
            "wk": pack_w(np.ascontiguousarray(w_k[sl, :].T), NKB),
            "wv": pack_w(np.ascontiguousarray(w_v[sl, :].T), NKB),
            "wo": pack_w(np.ascontiguousarray(w_o[:, sl].T), 2),
            "bq": np.ascontiguousarray(b_q[sl].astype(np.float32).reshape(2, 128).T),
            "bk": np.ascontiguousarray(b_k[sl].astype(np.float32).reshape(2, 128).T),
            "bvb": np.ascontiguousarray(
                np.broadcast_to(b_v[sl].astype(np.float32), (128, DSH))),
            "bob": bob,
            "mb": mb_b[b],
        })
    return in_maps


_NC_CACHE = {}


def kernel(q_in, k_in, v_in, mask, w_q, b_q, w_k, b_k, w_v, b_v, w_o, b_o):
    q_in, k_in, v_in, mask = (np.asarray(a) for a in (q_in, k_in, v_in, mask))
    w_q, b_q, w_k, b_k = (np.asarray(a) for a in (w_q, b_q, w_k, b_k))
    w_v, b_v, w_o, b_o = (np.asarray(a) for a in (w_v, b_v, w_o, b_o))
    if "nc" not in _NC_CACHE:
        _NC_CACHE["nc"] = build_nc()
    nc = _NC_CACHE["nc"]
    in_maps = _prep_inputs(q_in, k_in, v_in, mask,
                           w_q, b_q, w_k, b_k, w_v, b_v, w_o, b_o)
    res = run_bass_kernel_spmd(nc, in_maps, list(range(DP * TP))).results
    full = np.empty((B, S, D), np.float32)
    for b in range(B):
        for r in range(TP):
            o = res[TP * b + r]["out"]          # [NQB, 128, D]
            for qb in range(NQB):
                row = qb * QB + r * 128
                full[b, row:row + 128] = o[qb]
    return full


# revision 36
# speedup vs baseline: 404.2782x; 1.0092x over previous
"""Multi-head attention (B=2, S=2048, D=1024, H=16) on 8 TRN2 NeuronCores.

Sharding: tensor-parallel over heads (TP=4, 4 heads / 256 dims per core)
x data-parallel over batch (DP=2). Core c = 4*b + t handles batch b,
head group t. Each core computes Q/K/V projections for its heads,
attention in a transposed-scores layout (scores^T = [s_k, s_q], softmax
across partitions via a ones-column appended to V and a K=1 outer-product
broadcast of the reciprocal), then its partial output projection.
Partials are ReduceScattered over each batch's 4-core TP group; the host
reassembles the full [B, S, D] output.

All matmul operands are bf16 (fp32 PSUM accumulation); softmax
denominators/reciprocals and the output path are fp32. The key mask is
folded into the exp as a per-partition bias (0 or -60).
"""

import contextlib
import numpy as np
import ml_dtypes

import concourse.bass as bass
import concourse.tile as tile
from concourse import bacc, mybir
from concourse.bass_utils import run_bass_kernel_spmd

F32 = mybir.dt.float32
BF16 = mybir.dt.bfloat16
Exp = mybir.ActivationFunctionType.Exp

B, S, D, H = 2, 2048, 1024, 16
DK = D // H                      # 64
TP, DP = 4, 2
HPC = H // TP                    # heads per core = 4
DSH = D // TP                    # shard dims per core = 256
NPAIR = HPC // 2                 # head pairs per core = 2
QB = 512                         # query block
NQB = S // QB                    # 4
KT = 128                         # key tile
NKT = S // KT                    # 16
NKB = D // 128                   # 8 contraction tiles for projections
MASK_NEG = -60.0

REPLICA_GROUPS = [[0, 1, 2, 3], [4, 5, 6, 7]]


def build_nc(with_collective=True):
    nc = bacc.Bacc("TRN2", target_bir_lowering=False, debug=False, num_devices=DP * TP)

    # ---- parameters (per-core shards, host-prepped layouts)
    xq = nc.declare_dram_parameter("xq", [NKB, 128, S], BF16, isOutput=False)   # q_in[b].T
    xk = nc.declare_dram_parameter("xk", [NKB, 128, S], BF16, isOutput=False)
    xv = nc.declare_dram_parameter("xv", [NKB, 128, S], BF16, isOutput=False)
    # weights pre-packed on host into the exact SBUF layout -> 1 DMA each
    wq = nc.declare_dram_parameter("wq", [128, NKB * DSH], BF16, isOutput=False)
    wk = nc.declare_dram_parameter("wk", [128, NKB * DSH], BF16, isOutput=False)
    wv = nc.declare_dram_parameter("wv", [128, NKB * DSH], BF16, isOutput=False)
    wo = nc.declare_dram_parameter("wo", [128, 2 * D], BF16, isOutput=False)
    bq = nc.declare_dram_parameter("bq", [128, 2], F32, isOutput=False)
    bk = nc.declare_dram_parameter("bk", [128, 2], F32, isOutput=False)
    bvb = nc.declare_dram_parameter("bvb", [128, DSH], F32, isOutput=False)      # b_v shard bcast
    bob = nc.declare_dram_parameter("bob", [128, D], F32, isOutput=False)        # b_o bcast
    mb = nc.declare_dram_parameter("mb", [128, NKT], F32, isOutput=False)        # mask bias
    out = nc.declare_dram_parameter("out", [NQB, 128, D], F32, isOutput=True)

    with tile.TileContext(nc) as tc, contextlib.ExitStack() as ctx:
        const = ctx.enter_context(tc.tile_pool(name="const", bufs=1))
        xp = ctx.enter_context(tc.tile_pool(name="xp", bufs=3 * NKB))
        qt_p = ctx.enter_context(tc.tile_pool(name="qt", bufs=2 * NQB))
        kt_p = ctx.enter_context(tc.tile_pool(name="ktp", bufs=2 * NQB))
        vp_p = ctx.enter_context(tc.tile_pool(name="vp", bufs=NKT))
        exp_p = ctx.enter_context(tc.tile_pool(name="expp", bufs=6))
        ctx_p = ctx.enter_context(tc.tile_pool(name="ctxp", bufs=4))
        rec_p = ctx.enter_context(tc.tile_pool(name="recp", bufs=3))
        rb_p = ctx.enter_context(tc.tile_pool(name="rbp", bufs=2))
        po_p = ctx.enter_context(tc.tile_pool(name="pop", bufs=3))
        os_p = ctx.enter_context(tc.tile_pool(name="osp", bufs=2))
        ps_s = ctx.enter_context(tc.tile_pool(name="pss", bufs=2, space="PSUM"))
        ps_av = ctx.enter_context(tc.tile_pool(name="psav", bufs=2, space="PSUM"))
        ps_sm = ctx.enter_context(tc.tile_pool(name="pssm", bufs=2, space="PSUM"))
        dram = ctx.enter_context(tc.tile_pool(name="dram", bufs=2, space="DRAM"))

        # ---- constants (each one contiguous DMA; ordered by first use)
        w_sb = {name: const.tile([128, NKB * DSH], BF16, name=f"{name}_sb")
                for name in ("wk", "wv", "wq")}
        wo_sb = const.tile([128, 2 * D], BF16)
        bq_sb = const.tile([128, 2], F32)
        bk_sb = const.tile([128, 2], F32)
        bvb_sb = const.tile([128, DSH], F32)
        bob_sb = const.tile([128, D], F32)
        mb_sb = const.tile([128, NKT], F32)
        ones_sb = const.tile([128, DK], F32)
        nc.sync.dma_start(out=w_sb["wk"][:], in_=wk[:])
        nc.scalar.dma_start(out=w_sb["wv"][:], in_=wv[:])
        nc.scalar.dma_start(out=mb_sb[:], in_=mb[:])
        nc.any.memset(ones_sb[:], 1.0)

        # ---- phase A: projections
        # K^T and Q^T per (pair m, s-block nb): tiles [128, 512]
        #   partitions 0:64 = head 2m dims, 64:128 = head 2m+1 dims
        # V' per s-tile st: [128, HPC*65] with ones col at 64 of each 65
        KT_t = {}
        QT_t = {}
        VP_t = {}

        _xt_cache = {}

        def load_x(xprm, wname, eng, chunked=False):
            xt = [xp.tile([128, S], BF16, name=f"x_{wname}_{kb}", tag="xtile")
                  for kb in range(NKB)]
            if chunked:
                # nb-major: the first projection chain only needs every
                # tile's first chunk, so it can start ~3x earlier
                for nb in range(NQB):
                    for kb in range(NKB):
                        eng.dma_start(out=xt[kb][:, nb * QB:(nb + 1) * QB],
                                      in_=xprm[kb, :, nb * QB:(nb + 1) * QB])
            else:
                for kb in range(NKB):
                    eng.dma_start(out=xt[kb][:], in_=xprm[kb])
            _xt_cache[wname] = xt

        def proj_qk_chain(wname, bias_sb, store, nb, m):
            xt = _xt_cache[wname]
            ps = ps_sm.tile([128, QB], F32, name=f"ps_{wname}_{m}_{nb}", tag="smps")
            for kb in range(NKB):
                nc.tensor.matmul(
                    ps[:],
                    w_sb[wname][:, kb * DSH + m * 128: kb * DSH + (m + 1) * 128],
                    xt[kb][:, nb * QB:(nb + 1) * QB],
                    start=(kb == 0), stop=(kb == NKB - 1),
                )
            dst = (qt_p if store is QT_t else kt_p).tile(
                [128, QB], BF16, name=f"{wname}t_{m}_{nb}", tag="proj")
            nc.vector.tensor_scalar_add(dst[:], ps[:], bias_sb[:, m:m + 1])
            store[(m, nb)] = dst

        def proj_v_chain(st):
            xt = _xt_cache["wv"]
            ps = ps_sm.tile([128, QB], F32, name=f"ps_v_{st}", tag="smps")[:, 0:DSH]
            for kb in range(NKB):
                nc.tensor.matmul(
                    ps[:],
                    xt[kb][:, st * 128:(st + 1) * 128],
                    w_sb["wv"][:, kb * DSH:(kb + 1) * DSH],
                    start=(kb == 0), stop=(kb == NKB - 1),
                )
            vp = vp_p.tile([128, HPC * (DK + 1)], BF16, name=f"vp_{st}", tag="vp")
            for h in range(HPC):
                col = h * (DK + 1) + DK
                nc.any.memset(vp[:, col:col + 1], 1.0)
            ps3 = ps.rearrange("p (h d) -> p h d", h=HPC)
            bv3 = bvb_sb.rearrange("p (h d) -> p h d", h=HPC)
            vp3 = vp.rearrange("p (h d) -> p h d", h=HPC)[:, :, 0:DK]
            nc.vector.tensor_add(vp3, ps3, bv3)
            VP_t[st] = vp

        # x loads: xk full tiles + xq split (nb0 chunk early, rest later)
        # on the sync ring; xv full tiles behind the K/V weights on the
        # scalar ring. Late consts trail on the scalar ring.
        xt_k = [xp.tile([128, S], BF16, name=f"x_wk_{kb}", tag="xtile")
                for kb in range(NKB)]
        xt_v = [xp.tile([128, S], BF16, name=f"x_wv_{kb}", tag="xtile")
                for kb in range(NKB)]
        xt_q = [xp.tile([128, S], BF16, name=f"x_wq_{kb}", tag="xtile")
                for kb in range(NKB)]
        _xt_cache.update(wk=xt_k, wv=xt_v, wq=xt_q)
        for kb in range(NKB):
            nc.sync.dma_start(out=xt_k[kb][:], in_=xk[kb])
        for kb in range(NKB):
            nc.scalar.dma_start(out=xt_v[kb][:], in_=xv[kb])
        nc.scalar.dma_start(out=bk_sb[:], in_=bk[:])
        nc.scalar.dma_start(out=bvb_sb[:], in_=bvb[:])
        for kb in range(NKB):
            nc.sync.dma_start(out=xt_q[kb][:, 0:QB], in_=xq[kb, :, 0:QB])
        nc.scalar.dma_start(out=w_sb["wq"][:], in_=wq[:])
        nc.scalar.dma_start(out=bq_sb[:], in_=bq[:])
        for kb in range(NKB):
            nc.sync.dma_start(out=xt_q[kb][:, QB:S], in_=xq[kb, :, QB:S])
        nc.scalar.dma_start(out=wo_sb[:], in_=wo[:])
        nc.scalar.dma_start(out=bob_sb[:], in_=bob[:])

        for nb in range(NQB):
            for m in range(2):
                proj_qk_chain("wk", bk_sb, KT_t, nb, m)
        for m in range(2):
            proj_qk_chain("wq", bq_sb, QT_t, 0, m)
        for st in range(2):
            proj_v_chain(st)

        # ---- phase B: attention + output projection + reduce-scatter
        for qb in range(NQB):
            ctx_pair = []
            for m in range(NPAIR):
                av = [ps_av.tile([128, QB], F32, name=f"av_{qb}_{m}_{p}", tag="av")
                      for p in range(2)]

                def emit_av(kt, ets):
                    for p in range(2):
                        h = 2 * m + p
                        nc.tensor.matmul(
                            av[p][0:DK + 1, :],
                            VP_t[kt][:, h * (DK + 1):(h + 1) * (DK + 1)],
                            ets[:, p * QB:(p + 1) * QB],
                            start=(kt == 0), stop=(kt == NKT - 1),
                        )

                # software pipeline: AV(kt-1) is emitted after scores(kt) so
                # the PE never head-of-line blocks on exp(kt)
                prev_et = None
                for kt in range(NKT):
                    nb, co = kt // 4, (kt % 4) * 128
                    pss = ps_s.tile([128, 2 * QB], F32, name=f"pss_{qb}_{m}_{kt}", tag="pss")
                    # head 2m on partitions 0:64, head 2m+1 on 64:128;
                    # different PSUM banks for the two row groups (HW req.)
                    nc.tensor.matmul(
                        pss[:, 0:QB],
                        KT_t[(m, nb)][0:64, co:co + 128],
                        QT_t[(m, qb)][0:64, :],
                        start=True, stop=True)
                    nc.tensor.matmul(
                        pss[:, QB:2 * QB],
                        KT_t[(m, nb)][64:128, co:co + 128],
                        QT_t[(m, qb)][64:128, :],
                        start=True, stop=True)
                    et = exp_p.tile([128, 2 * QB], BF16, name=f"exp_{qb}_{m}_{kt}", tag="exp")
                    nc.scalar.activation(et[:], pss[:], Exp,
                                         bias=mb_sb[:, kt:kt + 1], scale=1.0 / np.sqrt(DK))
                    if prev_et is not None:
                        emit_av(kt - 1, prev_et)
                    prev_et = et
                    # PE fillers inside the ACT-bound loop: remaining V' and
                    # K chains during (qb0, m0); next Q block during each m1.
                    # K(nb) is consumed from iteration 4*nb on; V'(st) from
                    # iteration st on.
                    if qb == 0 and m == 0:
                        if kt + 2 < NKT:
                            proj_v_chain(kt + 2)
                    if m == 1 and qb + 1 < NQB and kt in (4, 12):
                        proj_qk_chain("wq", bq_sb, QT_t, qb + 1, 0 if kt == 4 else 1)
                emit_av(NKT - 1, prev_et)
                cpt = ctx_p.tile([128, QB], BF16, name=f"ctx_{qb}_{m}", tag="ctx")
                for p in range(2):
                    rec = rec_p.tile([128, QB], F32, name=f"rec_{qb}_{m}_{p}", tag="rec")
                    nc.vector.reciprocal(rec[64:65, :], av[p][DK:DK + 1, :])
                    rbp = ps_sm.tile([128, QB], F32, name=f"rbp_{qb}_{m}_{p}", tag="smps")
                    nc.tensor.matmul(rbp[0:DK, :], ones_sb[64:65, :],
                                     rec[64:65, :], start=True, stop=True)
                    rbs = rb_p.tile([DK, QB], F32, name=f"rbs_{qb}_{m}_{p}", tag="rbs")
                    nc.vector.tensor_copy(rbs[:], rbp[0:DK, :])
                    nc.vector.tensor_mul(cpt[p * DK:(p + 1) * DK, :], av[p][0:DK, :], rbs[:])
                ctx_pair.append(cpt)

            partial = dram.tile([QB, D], F32, name=f"partial_{qb}", tag="partial")
            for st in range(NQB):
                for dh in range(2):
                    pso = ps_sm.tile([128, 512], F32, name=f"pso_{qb}_{st}_{dh}", tag="smps")
                    for m in range(NPAIR):
                        nc.tensor.matmul(
                            pso[:],
                            ctx_pair[m][:, st * 128:(st + 1) * 128],
                            wo_sb[:, m * D + dh * 512: m * D + (dh + 1) * 512],
                            start=(m == 0), stop=(m == NPAIR - 1),
                        )
                    pos = po_p.tile([128, 512], F32, name=f"pos_{qb}_{st}_{dh}", tag="pos")
                    nc.vector.tensor_copy(pos[:], pso[:])
                    nc.sync.dma_start(
                        out=partial[st * 128:(st + 1) * 128, dh * 512:(dh + 1) * 512],
                        in_=pos[:])

            rs_out = dram.tile([128, D], F32, name=f"rs_{qb}", tag="rs")
            if with_collective:
                nc.gpsimd.collective_compute(
                    "ReduceScatter", mybir.AluOpType.add,
                    replica_groups=REPLICA_GROUPS,
                    ins=[partial[:].opt()], outs=[rs_out[:].opt()])
            else:
                nc.sync.dma_start(out=rs_out[:], in_=partial[0:128, :])
            osb = os_p.tile([128, D], F32, name=f"os_{qb}", tag="os")
            nc.sync.dma_start(out=osb[:], in_=rs_out[:])
            fin = os_p.tile([128, D], F32, name=f"fin_{qb}", tag="fin")
            nc.vector.tensor_add(fin[:], osb[:], bob_sb[:])
            nc.sync.dma_start(out=out[qb], in_=fin[:])

    nc.compile()
    return nc


def _prep_inputs(q_in, k_in, v_in, mask, w_q, b_q, w_k, b_k, w_v, b_v, w_o, b_o):
    BF = ml_dtypes.bfloat16
    xq_b, xk_b, xv_b, mb_b = [], [], [], []
    for b in range(B):
        xq_b.append(np.ascontiguousarray(q_in[b].T).astype(BF).reshape(NKB, 128, S))
        xk_b.append(np.ascontiguousarray(k_in[b].T).astype(BF).reshape(NKB, 128, S))
        xv_b.append(np.ascontiguousarray(v_in[b].T).astype(BF).reshape(NKB, 128, S))
        mbias = ((mask[b, 0, 0, :] == 0) * np.float32(MASK_NEG)).astype(np.float32)
        mb_b.append(np.ascontiguousarray(mbias.reshape(NKT, 128).T))
    bob = np.ascontiguousarray(np.broadcast_to(b_o.astype(np.float32), (128, D)))
    in_maps = []
    for c in range(DP * TP):
        b, t = c // TP, c % TP
        sl = slice(DSH * t, DSH * (t + 1))
        def pack_w(w_t, nblk):
            # [d_in, cols] -> SBUF layout [128, nblk*cols]: block kb at
            # columns [kb*cols:(kb+1)*cols] holds d_in rows kb*128..+128
            cols = w_t.shape[1]
            return np.ascontiguousarray(
                w_t.reshape(nblk, 128, cols).transpose(1, 0, 2).reshape(128, nblk * cols)
            ).astype(BF)

        in_maps.append({
            "xq": xq_b[b], "xk": xk_b[b], "xv": xv_b[b],
            "wq": pack_w(np.ascontiguousarray(w_q[sl, :].T), NKB),
            "wk": pack_w(np.ascontiguousarray(w_k[sl, :].T), NKB),
            "wv": pack_w(np.ascontiguousarray(w_v[sl, :].T), NKB),
            "wo": pack_w(np.ascontiguousarray(w_o[:, sl].T), 2),
            "bq": np.ascontiguousarray(b_q[sl].astype(np.float32).reshape(2, 128).T),
            "bk": np.ascontiguousarray(b_k[sl].astype(np.float32).reshape(2, 128).T),
            "bvb": np.ascontiguousarray(
                np.broadcast_to(b_v[sl].astype(np.float32), (128, DSH))),
            "bob": bob,
            "mb": mb_b[b],
        })
    return in_maps


_NC_CACHE = {}


def kernel(q_in, k_in, v_in, mask, w_q, b_q, w_k, b_k, w_v, b_v, w_o, b_o):
    q_in, k_in, v_in, mask = (np.asarray(a) for a in (q_in, k_in, v_in, mask))
    w_q, b_q, w_k, b_k = (np.asarray(a) for a in (w_q, b_q, w_k, b_k))
    w_v, b_v, w_o, b_o = (np.asarray(a) for a in (w_v, b_v, w_o, b_o))
    if "nc" not in _NC_CACHE:
        _NC_CACHE["nc"] = build_nc()
    nc = _NC_CACHE["nc"]
    in_maps = _prep_inputs(q_in, k_in, v_in, mask,
                           w_q, b_q, w_k, b_k, w_v, b_v, w_o, b_o)
    res = run_bass_kernel_spmd(nc, in_maps, list(range(DP * TP))).results
    full = np.empty((B, S, D), np.float32)
    for b in range(B):
        for r in range(TP):
            o = res[TP * b + r]["out"]          # [NQB, 128, D]
            for qb in range(NQB):
                row = qb * QB + r * 128
                full[b, row:row + 128] = o[qb]
    return full
